# revision 34
# baseline (speedup 1.0000x reference)
"""GQA attention (B=1, S=2048, D=2048, 32 Q heads / 8 KV heads, RoPE, causal)
sharded tensor-parallel over KV-head groups across 8 NeuronCores.

Per core: 1 KV head + its 4 Q heads. Inputs are sequence/head sharded to
minimize per-execution IO bytes (the dominant per-dispatch runtime cost on
this backend):
  - x arrives sequence-sharded ([256, 2048] bf16 per core) and is
    AllGather'd on-device over NeuronLink (split in two chunks so the second
    gather overlaps the first chunk's compute).
  - cos/sin RoPE tables arrive sharded ([256, 128] f32) and are AllGather'd.
  - QKV projection: x^T tiles made on PE (transpose), qkv = x @ Wqkv^T via
    lhsT = x^T tile [d,128s], rhs = WqkvT [d, 384] -> psum [s=128, 384].
  - RoPE applied in natural layout with on-device 5-way table reuse.
  - Attention computed score-transposed: S^T[t, sq] = K_rot @ Q_rot^T so the
    probs land in the [t, sq] layout that P@V needs, the softmax denominator
    comes free as a 65th "ones" column appended to V, and causality skips
    whole tiles (only one triangular 128x128 mask per diag).
  - Output projection with the per-core 256-wide slice of wo -> partial
    [2048, 2048]; ReduceScatter sums the 8 partials on-device and leaves
    each core its own [256, 2048] output slice (the TP all-reduce).

The body can be emitted `reps` times in one NEFF so benchmark harnesses can
amortize per-dispatch runtime overhead; every repeat redoes all DMA,
collectives and compute (no cross-repeat reuse). kernel() uses reps=1.
"""

import sys

for _p in ("/opt/trn_rl_repo",):
    if _p not in sys.path:
        sys.path.insert(0, _p)

import ml_dtypes
import numpy as np

import concourse.bacc as bacc
import concourse.bass as bass
import concourse.mybir as mybir
import concourse.tile as tile
from concourse.bass_utils import run_bass_kernel_spmd
from concourse.masks import make_identity, make_upper_triangular

F32 = mybir.dt.float32
BF16 = mybir.dt.bfloat16

B, S, DIM = 1, 2048, 2048
NH, NKV, HD = 32, 8, 64
NHPC = NH // NKV          # q heads per core = 4
QSH = NHPC * HD           # q cols per core = 256
KVW = HD                  # kv cols per core = 64
QKVW = QSH + 2 * KVW      # fused qkv width = 384
NCORES = 8
P = 128
NS = S // P               # 16 s-chunks of 128
SSH = S // NCORES         # sequence shard rows per core = 256
SQT = 512                 # sq tile width for attention/wo
NJ = S // SQT             # 4 sq tiles
SCALE = HD ** -0.5
GROUPS = [list(range(NCORES))]


def _body(tc, ctx, reps=1):
    nc = tc.nc
    xs = nc.dram_tensor("xs", [SSH, DIM], BF16, kind="ExternalInput")
    wqkvt = nc.dram_tensor("wqkvt", [DIM, QKVW], BF16, kind="ExternalInput")
    wot = nc.dram_tensor("wot", [QSH, DIM], BF16, kind="ExternalInput")
    css = nc.dram_tensor("css", [S, 2 * HD], F32, kind="ExternalInput")
    out = nc.dram_tensor("out", [SSH, DIM], BF16, kind="ExternalOutput")

    QKW = QKVW - KVW  # 320: q(256) + k(64), rope'd together

    consts = ctx.enter_context(tc.tile_pool(name="consts", bufs=1))
    ident = consts.tile([P, P], BF16)
    make_identity(nc, ident[:])
    m01 = consts.tile([P, P], F32)  # m01[t, r] = 1 if r >= t else 0
    make_upper_triangular(nc, m01[:], val=1.0, diag=True)
    onesp = consts.tile([P, HD], BF16)
    nc.gpsimd.memset(onesp[:], 1.0)

    dram = ctx.enter_context(tc.tile_pool(name="dram", bufs=2, space="DRAM"))
    wts = ctx.enter_context(tc.tile_pool(name="wts", bufs=2))
    acts = ctx.enter_context(tc.tile_pool(name="acts", bufs=2))

    ps_tr = ctx.enter_context(tc.tile_pool(name="ps_tr", bufs=2, space="PSUM"))
    ps_mm = ctx.enter_context(tc.tile_pool(name="ps_mm", bufs=2, space="PSUM"))
    ps_acc = ctx.enter_context(tc.tile_pool(name="ps_acc", bufs=4, space="PSUM"))

    xn_pool = ctx.enter_context(tc.tile_pool(name="xn", bufs=2))
    cs_pool = ctx.enter_context(tc.tile_pool(name="cs", bufs=2))
    xt_pool = ctx.enter_context(tc.tile_pool(name="xt", bufs=4))
    qk_pool = ctx.enter_context(tc.tile_pool(name="qk", bufs=2))
    et_pool = ctx.enter_context(tc.tile_pool(name="et", bufs=3))
    sm_pool = ctx.enter_context(tc.tile_pool(name="sm", bufs=4))
    ob_pool = ctx.enter_context(tc.tile_pool(name="ob", bufs=4))
    os_pool = ctx.enter_context(tc.tile_pool(name="os", bufs=3))

    def emit_gather():
        # ---- transpose own x rows on PE, then gather x^T on-device ----
        # xt_b row-block h = own s-chunk h transposed, stored SBUF-verbatim as
        # [128 d-part, (dchunk, s)] so every DMA row is 4KB contiguous.
        # AllGather block g of xgt{h} is core g's chunk, global i = 2g+h.
        # Two gathers (not one) so the second's link time overlaps the first
        # half's stage-A compute.
        xt_b = dram.tile([2 * P, NS * P], BF16, tag="xtb")
        xgt0 = dram.tile([NCORES * P, NS * P], BF16, tag="xgt0")
        xgt1 = dram.tile([NCORES * P, NS * P], BF16, tag="xgt1")
        for h in (0, 1):
            xn = xn_pool.tile([P, DIM], BF16, tag="xn")
            nc.sync.dma_start(out=xn[:], in_=xs[P * h : P * (h + 1), :])
            xto = xt_pool.tile([P, NS * P], BF16, tag="xto")
            for d in range(NS):
                tp = ps_tr.tile([P, P], BF16, tag="tr")
                nc.tensor.matmul(tp[:], xn[:, P * d : P * (d + 1)], ident[:],
                                 is_transpose=True)
                nc.any.tensor_copy(xto[:, P * d : P * (d + 1)], tp[:])
            nc.sync.dma_start(out=xt_b[P * h : P * (h + 1), :], in_=xto[:])
        nc.gpsimd.collective_compute(
            "AllGather", mybir.AluOpType.bypass, GROUPS,
            ins=[xt_b[0:P, :].opt()], outs=[xgt0[:].opt()],
        )
        nc.gpsimd.collective_compute(
            "AllGather", mybir.AluOpType.bypass, GROUPS,
            ins=[xt_b[P : 2 * P, :].opt()], outs=[xgt1[:].opt()],
        )
        return xgt0, xgt1

    # software-pipeline the gathers one repeat ahead: repeat r+1's AllGathers
    # are issued on the collective queue BEFORE repeat r's ReduceScatter, so
    # the link transfers overlap repeat r's attention compute.
    nxt = emit_gather()
    for _rep in range(reps):
        xgt0, xgt1 = nxt
        if _rep + 1 < reps:
            nxt = emit_gather()
        prt = dram.tile([S, DIM], BF16, tag="prt")
        outb = dram.tile([SSH, DIM], BF16, tag="outb")

        # ---- per-repeat resident weights / activations ----
        wq_sb = wts.tile([P, NS * QKVW], BF16, tag="wq")  # [d-part, (dchunk, qkv)]
        nc.sync.dma_start(
            out=wq_sb[:].rearrange("p (c q) -> p c q", c=NS),
            in_=wqkvt[:].rearrange("(c p) q -> p c q", p=P),
        )
        wot_sb0 = wts.tile([P, DIM], BF16, tag="wo0")
        wot_sb1 = wts.tile([P, DIM], BF16, tag="wo1")
        nc.sync.dma_start(out=wot_sb0[:], in_=wot[0:P, :])
        nc.sync.dma_start(out=wot_sb1[:], in_=wot[P : 2 * P, :])

        qt01 = acts.tile([P, S], BF16, tag="qt01")  # heads 0,1 stacked on partitions
        qt23 = acts.tile([P, S], BF16, tag="qt23")  # heads 2,3
        kt2 = acts.tile([P, S], BF16, tag="kt2")    # k^T replicated on both halves
        vones = acts.tile([P, NS * (HD + 1)], BF16, tag="vones")
        nc.gpsimd.memset(vones[:], 1.0)

        # ---- stage A: projections + RoPE + transposes, per 128-row s-chunk ----
        for half in (0, 1):
            xgh = xgt0 if half == 0 else xgt1
            for g in range(NCORES):
                i = 2 * g + half  # global s-chunk index
                xtile = xt_pool.tile([P, NS * P], BF16, tag="xto")
                nc.sync.dma_start(out=xtile[:], in_=xgh[P * g : P * (g + 1), :])
                csb = cs_pool.tile([P, 2 * HD], F32, tag="cs")
                nc.sync.dma_start(out=csb[:], in_=css[P * i : P * (i + 1), :])
                cosb = csb[:, 0:HD]
                sinb = csb[:, HD : 2 * HD]

                qkvp = ps_mm.tile([P, QKVW], F32, tag="mm")
                for d in range(NS):
                    nc.tensor.matmul(
                        qkvp[:], xtile[:, P * d : P * (d + 1)],
                        wq_sb[:, d * QKVW : (d + 1) * QKVW],
                        start=(d == 0), stop=(d == NS - 1),
                    )

                # v chunk -> vones (65th col stays 1.0 from the memset)
                nc.any.tensor_copy(
                    vones[:, i * (HD + 1) : i * (HD + 1) + HD], qkvp[:, QKW:QKVW]
                )
                # rope on q+k block [128, 320]: 5 head-blocks share the tables
                qk = qk_pool.tile([P, QKW], F32, tag="qk")
                nc.any.tensor_copy(qk[:], qkvp[:, 0:QKW])
                qkv_pairs = qk[:].rearrange("p (g two) -> p g two", two=2)
                shuf = qk_pool.tile([P, QKW], F32, tag="shuf")
                shuf_pairs = shuf[:].rearrange("p (g two) -> p g two", two=2)
                nc.vector.tensor_copy(shuf_pairs[:, :, 0], qkv_pairs[:, :, 1])
                nc.vector.tensor_copy(shuf_pairs[:, :, 1], qkv_pairs[:, :, 0])
                rot = qk_pool.tile([P, QKW], BF16, tag="rot")
                for b5 in range(QKW // HD):
                    sl = slice(HD * b5, HD * (b5 + 1))
                    nc.vector.tensor_mul(rot[:, sl], qk[:, sl], cosb)
                    nc.vector.tensor_mul(shuf[:, sl], shuf[:, sl], sinb)
                nc.vector.tensor_add(rot[:], rot[:], shuf[:])

                # transpose rot -> qT / kT
                for (lo, dst) in ((0, qt01), (P, qt23)):
                    tq = ps_tr.tile([P, P], BF16, tag="tr")
                    nc.tensor.matmul(tq[:], rot[:, lo : lo + P], ident[:],
                                     is_transpose=True)
                    nc.any.tensor_copy(dst[:, P * i : P * (i + 1)], tq[:])
                tk = ps_tr.tile([HD, P], BF16, tag="tr")
                nc.tensor.matmul(tk[:], rot[:, 2 * P : 2 * P + HD], ident[:],
                                 is_transpose=True)
                nc.any.tensor_copy(kt2[0:HD, P * i : P * (i + 1)], tk[:])
                nc.any.tensor_copy(kt2[HD:P, P * i : P * (i + 1)], tk[:])

        # ---- stage B: attention + wo, per 512-wide sq tile ----
        for j in range(NJ):
            ncv = 4 * (j + 1)  # t-chunks this sq tile sees
            ovp = [
                ps_acc.tile([HD + 1, SQT], F32, tag="acc", name=f"ovp{_rep}_{j}_{h}")
                for h in range(NHPC)
            ]
            for c in range(ncv):
                c0 = max(0, P * c - SQT * j)
                w = SQT - c0
                for h in range(NHPC):
                    qt = qt01 if h < 2 else qt23
                    pb = HD * (h % 2)
                    sp = ps_tr.tile([P, w], F32, tag="tr")
                    nc.tensor.matmul(
                        sp[:],
                        kt2[pb : pb + HD, P * c : P * (c + 1)],
                        qt[pb : pb + HD, SQT * j + c0 : SQT * (j + 1)],
                    )
                    et = et_pool.tile([P, w], BF16, tag="et")
                    nc.scalar.activation(
                        et[:], sp[:], mybir.ActivationFunctionType.Exp, scale=SCALE
                    )
                    if P * c >= SQT * j:  # diagonal chunk: triangular mask
                        nc.any.tensor_mul(et[:, 0:P], et[:, 0:P], m01[:])
                    nc.tensor.matmul(
                        ovp[h][:, c0:SQT],
                        vones[:, c * (HD + 1) : (c + 1) * (HD + 1)],
                        et[:],
                        start=(c == 0), stop=(c == ncv - 1),
                    )

            osb01 = ob_pool.tile([P, SQT], BF16, tag="ob")
            osb23 = ob_pool.tile([P, SQT], BF16, tag="ob")
            for h in range(NHPC):
                rc = sm_pool.tile([P, SQT], BF16, tag="rc")
                nc.vector.reciprocal(rc[HD : HD + 1, :], ovp[h][HD : HD + 1, :])
                rp = ps_tr.tile([HD, SQT], F32, tag="tr")
                nc.tensor.matmul(
                    rp[:], onesp[HD : HD + 1, 0:HD], rc[HD : HD + 1, :],
                    tile_position=(HD, 0),
                )
                dst = osb01 if h < 2 else osb23
                lo = HD * (h % 2)
                nc.any.tensor_copy(dst[lo : lo + HD, :], ovp[h][0:HD, :])
                nc.any.tensor_mul(dst[lo : lo + HD, :], dst[lo : lo + HD, :], rp[:])

            for m in range(SQT // P):
                ob = os_pool.tile([P, DIM], BF16, tag="os")
                for e in range(DIM // SQT):
                    wp = ps_mm.tile([P, SQT], F32, tag="mm")
                    nc.tensor.matmul(
                        wp[:], osb01[:, P * m : P * (m + 1)],
                        wot_sb0[:, SQT * e : SQT * (e + 1)],
                        start=True, stop=False,
                    )
                    nc.tensor.matmul(
                        wp[:], osb23[:, P * m : P * (m + 1)],
                        wot_sb1[:, SQT * e : SQT * (e + 1)],
                        start=False, stop=True,
                    )
                    nc.any.tensor_copy(ob[:, SQT * e : SQT * (e + 1)], wp[:])
                nc.scalar.dma_start(
                    out=prt[SQT * j + P * m : SQT * j + P * (m + 1), :],
                    in_=ob[:],
                )

        # ---- on-device TP all-reduce: each core keeps its sequence slice ----
        nc.gpsimd.collective_compute(
            "ReduceScatter", mybir.AluOpType.add, GROUPS,
            ins=[prt[:].opt()], outs=[outb[:].opt()],
        )
        nc.sync.dma_start(out=out[:], in_=outb[:])


_CACHE = {}


def _build(reps=1):
    key = f"nc{reps}"
    if key not in _CACHE:
        from contextlib import ExitStack

        nc = bacc.Bacc(None, target_bir_lowering=False)
        with tile.TileContext(nc) as tc, ExitStack() as ctx:
            with nc.allow_low_precision(reason="bf16 matmul pipeline"):
                _body(tc, ctx, reps=reps)
        nc.compile()
        _CACHE[key] = nc
    return _CACHE[key]


def _host_tables(freqs_cis):
    # cos/sin tables in natural [s, col] layout for one 64-wide head block:
    # cols 2i/2i+1 <- cos_i/cos_i and -sin_i/+sin_i.
    cos = freqs_cis[..., 0].astype(np.float32)  # (S, 32)
    sin = freqs_cis[..., 1].astype(np.float32)
    cos2 = np.repeat(cos, 2, axis=1)            # (S, 64)
    sin2 = np.empty_like(cos2)
    sin2[:, 0::2] = -sin                        # even: -sin
    sin2[:, 1::2] = sin                         # odd:  +sin
    return np.ascontiguousarray(np.concatenate([cos2, sin2], axis=1))  # (S, 128)


def _in_maps(x, wq, wk, wv, wo, freqs_cis):
    bf = ml_dtypes.bfloat16
    xf = np.ascontiguousarray(np.asarray(x, np.float32)[0].astype(bf))  # (S, DIM)
    css = _host_tables(np.asarray(freqs_cis))
    in_maps = []
    for c in range(NCORES):
        wq_c = np.asarray(wq, np.float32)[c * QSH : (c + 1) * QSH]   # (256, D)
        wk_c = np.asarray(wk, np.float32)[c * KVW : (c + 1) * KVW]   # (64, D)
        wv_c = np.asarray(wv, np.float32)[c * KVW : (c + 1) * KVW]
        wqkvt = np.ascontiguousarray(
            np.concatenate([wq_c, wk_c, wv_c], axis=0).T.astype(bf)  # (D, 384)
        )
        wot = np.ascontiguousarray(
            np.asarray(wo, np.float32)[:, c * QSH : (c + 1) * QSH].T.astype(bf)
        )
        in_maps.append(
            dict(
                xs=np.ascontiguousarray(xf[c * SSH : (c + 1) * SSH]),
                wqkvt=wqkvt,
                wot=wot,
                css=css,
            )
        )
    return in_maps


def kernel(x, wq, wk, wv, wo, freqs_cis, mask):
    nc = _build(reps=1)
    in_maps = _in_maps(x, wq, wk, wv, wo, freqs_cis)
    res = run_bass_kernel_spmd(nc, in_maps, list(range(NCORES)))
    _CACHE["last"] = res
    full = np.concatenate(
        [np.asarray(res.results[c]["out"]).astype(np.float32) for c in range(NCORES)],
        axis=0,
    )
    return full.reshape(B, S, DIM)


if __name__ == "__main__":
    _build()
    print("build ok")
